# revision 4
# baseline (speedup 1.0000x reference)
"""Maxwell rheological model kernel for Trainium2 (8 NeuronCores, SPMD).

Recurrence per batch row (a = E/ETA = 2, E_INFTY = 1, E = 2):
    gamma[0] = 0
    gamma[n+1] = (1 - 2*dt[n]) * gamma[n] + 2*dt[n] * eps[n]
    sigma[n+1] = 3*eps[n+1] - 2*gamma[n+1];  sigma[0] = 0

Scaled form used on device (G' = 2*gamma/3, S = sigma/3; host multiplies
the output by 3):
    c[n]  = 1 - 2*dt[n]                     (f32, ACT, dequants int16 dt)
    t'[n] = (4/3)*dt[n]                     (f32, ACT)
    d[n]  = t'[n] * eps[n]                  (f32, POOL tensor_tensor)
    G'[n] = c[n]*G'[n-1] + d[n]             (DVE scan, f32 data, f16 out)
    S[m]  = eps[m] - G'[m-1]                (f16 tensor_tensor, 2x mode)

The scan output is staged one column later in a padded G tile whose
column 0 holds eps[0]; the sigma subtract then runs fully aligned from
column 0 and S[0] = eps[0] - eps[0] = 0 exactly.

Engine budget per core (measured rates): scan 69us is the DVE floor;
the d-product lives on the otherwise idle Pool engine, c/t' on ACT.
All DMA moves 2-byte elements (int16 dt, f16 eps/out) = 25 MB/core.
Batch rows are packed two-per-partition ([1024, 4096] view of the
per-core [2048, 2048] array) so DMAs move contiguous 1 MiB blocks.
Batch is sharded across 8 cores (data parallel, no collectives).
"""

import sys

if "/opt/trn_rl_repo" not in sys.path:
    sys.path.insert(0, "/opt/trn_rl_repo")

import numpy as np

import concourse.bacc as bacc
import concourse.mybir as mybir
from concourse.bass_utils import run_bass_kernel_spmd
from concourse.tile import TileContext

B, T = 16384, 2048
N_CORES = 8
B_CORE = B // N_CORES
P = 128
R = B_CORE // 2          # packed rows per core (2 batch rows / partition)
F = 2 * T                # packed free size
GF = F + 4               # G tile: +1 pad col per half, rounded to keep
                         # half 1's section 4B-aligned (offset T+2)
N_STRIPS = R // P        # 8

S_DT = 1.0 / 32767.0

# sigma-subtract jobs routed to Pool instead of DVE, as (strip, half)
SIGMA_POOL_JOBS: set = set()

_prog = None


def _build():
    f16 = mybir.dt.float16
    f32 = mybir.dt.float32
    i16 = mybir.dt.int16
    Alu = mybir.AluOpType
    Act = mybir.ActivationFunctionType
    nc = bacc.Bacc(
        "TRN2",
        target_bir_lowering=False,
        debug=False,
        enable_asserts=False,
    )
    qdt = nc.dram_tensor("qdt", [R, F], i16, kind="ExternalInput").ap()
    eps = nc.dram_tensor("eps", [R, F], f16, kind="ExternalInput").ap()
    out = nc.dram_tensor("out", [R, F], f16, kind="ExternalOutput").ap()
    with TileContext(nc) as tc:
        with (
            tc.tile_pool(name="pin", bufs=3) as pin,
            tc.tile_pool(name="pmid", bufs=2) as pmid,
            tc.tile_pool(name="pout", bufs=3) as pout,
        ):
            for s in range(N_STRIPS):
                r0 = s * P
                qd_t = pin.tile([P, F], i16, tag="qd")
                e_t = pin.tile([P, F], f16, tag="eps")
                c_t = pmid.tile([P, F], f32, tag="c")
                t_t = pmid.tile([P, F], f32, tag="t")
                d_t = pmid.tile([P, F], f32, tag="d")
                g_t = pmid.tile([P, GF], f16, tag="g")
                s_t = pout.tile([P, F], f16, tag="sig")

                # Loads: strip 0 is chunked so compute starts early.
                if s == 0:
                    lbounds = [0, 1024, 2048, F]
                else:
                    lbounds = [0, F]
                for lo, hi in zip(lbounds[:-1], lbounds[1:]):
                    nc.sync.dma_start(out=qd_t[:, lo:hi], in_=qdt[r0 : r0 + P, lo:hi])
                    nc.sync.dma_start(out=e_t[:, lo:hi], in_=eps[r0 : r0 + P, lo:hi])

                # Column chunking (chained scans) on first/last strips to
                # shorten the pipeline head and tail.
                chunked = s == 0 or s == N_STRIPS - 1
                for h in range(2):
                    o = h * T
                    go = h * (T + 2)
                    # G[0] = eps[0]  ->  S[0] = 0 exactly
                    nc.scalar.activation(
                        out=g_t[:, go : go + 1],
                        in_=e_t[:, o : o + 1],
                        func=Act.Copy,
                    )
                    bounds = [0, 1024, T] if chunked else [0, T]
                    for lo, hi in zip(bounds[:-1], bounds[1:]):
                        ch = min(hi, T - 1)
                        # ACT: c = 1 - 2*dt  (f32)
                        nc.scalar.activation(
                            out=c_t[:, o + lo : o + ch],
                            in_=qd_t[:, o + lo : o + ch],
                            func=Act.Copy,
                            scale=-2.0 * S_DT,
                            bias=1.0,
                        )
                        # ACT: t' = (4/3)*dt  (f32)
                        nc.scalar.activation(
                            out=t_t[:, o + lo : o + ch],
                            in_=qd_t[:, o + lo : o + ch],
                            func=Act.Copy,
                            scale=4.0 / 3.0 * S_DT,
                        )
                        # POOL: d = t' * eps  (f32)
                        nc.gpsimd.tensor_tensor(
                            out=d_t[:, o + lo : o + ch],
                            in0=t_t[:, o + lo : o + ch],
                            in1=e_t[:, o + lo : o + ch],
                            op=Alu.mult,
                        )
                        # DVE: G'[n] = c*G' + d, staged +1 col in g_t
                        nc.vector.tensor_tensor_scan(
                            out=g_t[:, go + lo + 1 : go + ch + 1],
                            data0=c_t[:, o + lo : o + ch],
                            data1=d_t[:, o + lo : o + ch],
                            initial=0.0 if lo == 0 else g_t[:, go + lo : go + lo + 1],
                            op0=Alu.mult,
                            op1=Alu.add,
                        )
                        # S[m] = eps[m] - G'[m-1]   (f16 TT, aligned cols)
                        eng = (
                            nc.gpsimd if (s, h) in SIGMA_POOL_JOBS else nc.vector
                        )
                        eng.tensor_tensor(
                            out=s_t[:, o + lo : o + hi],
                            in0=e_t[:, o + lo : o + hi],
                            in1=g_t[:, go + lo : go + hi],
                            op=Alu.subtract,
                        )
                        if s == N_STRIPS - 1:
                            nc.scalar.dma_start(
                                out=out[r0 : r0 + P, o + lo : o + hi],
                                in_=s_t[:, o + lo : o + hi],
                            )
                if s != N_STRIPS - 1:
                    nc.scalar.dma_start(out=out[r0 : r0 + P, :], in_=s_t)
    nc.compile()
    return nc


def _get_prog():
    global _prog
    if _prog is None:
        _prog = _build()
    return _prog


def _run(strains, dts, **kwargs):
    nc = _get_prog()
    qd = np.clip(
        np.rint(np.ascontiguousarray(dts, dtype=np.float32) * np.float32(1.0 / S_DT)),
        0,
        32767,
    ).astype(np.int16)
    ef = np.ascontiguousarray(strains, dtype=np.float32).astype(np.float16)
    qds = np.split(qd.reshape(N_CORES * R, F), N_CORES, axis=0)
    efs = np.split(ef.reshape(N_CORES * R, F), N_CORES, axis=0)
    in_maps = [{"qdt": d, "eps": e} for d, e in zip(qds, efs)]
    res = run_bass_kernel_spmd(nc, in_maps, core_ids=list(range(N_CORES)), **kwargs)
    full = np.concatenate([r["out"] for r in res.results], axis=0)
    full = full.reshape(B, T).astype(np.float32) * np.float32(3.0)
    return full, res


def kernel(strains, dts):
    out, _ = _run(strains, dts)
    return out


if __name__ == "__main__":
    rng = np.random.default_rng(0)
    eps = rng.standard_normal((B, T), dtype=np.float32)
    dts = rng.random((B, T), dtype=np.float32)
    out = kernel(eps, dts)
    print("ran ok", out.shape, out.dtype)
